# revision 8
# baseline (speedup 1.0000x reference)
"""Trainium2 Bass kernel for nn_GatherLayer (embedding_lookup).

Per sample b: out[b, :] = full_output[b, idx[b]*512 : (idx[b]+1)*512]

Strategy (pure data parallel across 8 NeuronCores):
  - Each core owns 2048 batch rows. Its slice of full_output is viewed as a
    [2048*18, 512] f32 table; the per-row action index idx[b] selects chunk
    row b_local*18 + idx[b].
  - On device, the SWDGE custom instruction InstDMAGatherAnt (nc.gpsimd.
    dma_gather) gathers 2KB rows from HBM into SBUF by int16 indices.
    Because int16 caps the index range at 32767 (< 2048*18=36864), the
    2048 rows are processed in chunks of CR rows, each gather reading from
    a chunk-local base of the table.
  - dma_gather writes gather position i to SBUF partition i%128, slot
    i//128.  The index stream is permuted host-side so that partition p
    ends up holding output rows p*RPP .. p*RPP+RPP-1 of the chunk
    contiguously -> the SBUF->HBM writeback is a fully contiguous
    (RPP*2KB)-per-partition HWDGE DMA.
  - Writebacks alternate between the two HWDGE rings (SP via nc.sync, ACT
    via nc.scalar) and overlap with subsequent gathers (SWDGE).

HBM traffic per core: 4MB scattered 2KB reads + 4MB contiguous writes.
"""

import contextlib

import numpy as np

import concourse.bacc as bacc
import concourse.mybir as mybir
from concourse.bass_utils import run_bass_kernel_spmd
from concourse.library_config import mlp

# Problem shape (hardcoded per contract).
B = 16384          # batch
A = 18             # nb actions
D = 512            # output dim per action
N_CORES = 8
BC = B // N_CORES  # rows per core = 2048

# Chunking: one dma_gather per chunk of CR rows (chunk-local indices must
# fit int16: CR*18 <= 32767).
CR = 512
# SWDGE descriptor-ring carveout bytes (throttles in-flight gather descs).
# 64KB holds ~4096 descriptors: two full CR=512 gathers (1024 desc-pairs
# each) can be in flight, so Q7 generation never stalls the SDMA drain.
SCRATCH = 65536
N_QUEUES = 1

_NC_CACHE = {}
LAST_RESULTS = None  # test.py introspection


def _params():
    return BC // CR, CR // 128  # (N_CHUNKS, RPP)


def _build_nc():
    N_CHUNKS, RPP = _params()
    nc = bacc.Bacc(
        "TRN2", dynamic_dma_scratch_size=SCRATCH, num_swdge_queues=N_QUEUES
    )
    table = nc.dram_tensor(
        "table", [BC * A, D], mybir.dt.float32, kind="ExternalInput"
    )
    idxs_hbm = nc.dram_tensor(
        "gidx", [128, BC // 16], mybir.dt.int16, kind="ExternalInput"
    )
    out_t = nc.dram_tensor(
        "out", [N_CHUNKS, 128, RPP, D], mybir.dt.float32, kind="ExternalOutput"
    )

    ccols = CR // 16  # idx columns per chunk

    with (
        nc.Block() as block,
        nc.sbuf_tensor("idxs_sbuf", [128, BC // 16], mybir.dt.int16) as idxs_sbuf,
        nc.semaphore("io") as io,
        nc.semaphore("gsem") as gsem,
        nc.semaphore("wsem") as wsem,
        nc.semaphore("wsem2") as wsem2,
    ):
        dsts = []
        stack = contextlib.ExitStack()
        for k in range(N_CHUNKS):
            dsts.append(
                stack.enter_context(
                    nc.sbuf_tensor(f"dst{k}", [128, RPP, D], mybir.dt.float32)
                )
            )

        # Writebacks alternate between the two HWDGE rings (SP and ACT) so
        # descriptor generation overlaps and SDMA round-robins more rings.
        n_sp = (N_CHUNKS + 1) // 2
        n_act = N_CHUNKS - n_sp

        @block.sync
        def _(sync):
            sync.dma_start(idxs_sbuf[:, :], idxs_hbm[:, :]).then_inc(io, 16)
            for k in range(0, N_CHUNKS, 2):
                sync.wait_ge(gsem, 16 * (k + 1))
                sync.dma_start(out_t[k], dsts[k][:, :, :]).then_inc(wsem, 16)
            sync.wait_ge(wsem, 16 * n_sp)

        @block.scalar
        def _(scalar):
            for k in range(1, N_CHUNKS, 2):
                scalar.wait_ge(gsem, 16 * (k + 1))
                scalar.dma_start(out_t[k], dsts[k][:, :, :]).then_inc(wsem2, 16)
            scalar.wait_ge(wsem2, 16 * n_act)

        @block.gpsimd
        def _(gpsimd):
            gpsimd.load_library(mlp)
            gpsimd.wait_ge(io, 16)
            for k in range(N_CHUNKS):
                gpsimd.dma_gather(
                    dsts[k][:, :, :],
                    table[k * CR * A : (k + 1) * CR * A, :],
                    idxs_sbuf[:, k * ccols : (k + 1) * ccols],
                    CR,
                    CR,
                    D,
                ).then_inc(gsem, 16)

        stack.close()

    nc.compile()
    return nc


def _get_nc():
    if "nc" not in _NC_CACHE:
        _NC_CACHE["nc"] = _build_nc()
    return _NC_CACHE["nc"]


def _make_gidx(actions_core: np.ndarray) -> np.ndarray:
    """Per-core gather-index plane [128, BC//16] int16.

    Chunk k's block (columns k*CR//16 ...) holds, at wrapped position
    [i%16, i//16], the chunk-local table row for gather position i, where
    gather position i is assigned output row (i%128)*RPP + i//128 of the
    chunk (so SBUF partition p holds rows p*RPP..p*RPP+RPP-1 contiguously).
    """
    N_CHUNKS, RPP = _params()
    i = np.arange(CR)
    r = (i % 128) * RPP + i // 128           # chunk-local output row
    blocks = []
    for k in range(N_CHUNKS):
        act = actions_core[k * CR : (k + 1) * CR]
        vals = (r * A + act[r]).astype(np.int16)     # chunk-local table row
        block = vals.reshape(CR // 16, 16).T         # [16, CR/16]
        blocks.append(np.tile(block, (8, 1)))        # replicate for Q7 cores
    return np.ascontiguousarray(np.concatenate(blocks, axis=1))


def kernel(full_output: np.ndarray, indices: np.ndarray) -> np.ndarray:
    global LAST_RESULTS
    full_output = np.ascontiguousarray(np.asarray(full_output, dtype=np.float32))
    indices = np.asarray(indices, dtype=np.int32)
    assert full_output.shape == (B, A * D)
    assert indices.shape == (B, 1)

    nc = _get_nc()

    in_maps = []
    for c in range(N_CORES):
        sl = slice(c * BC, (c + 1) * BC)
        in_maps.append(
            {
                "table": full_output[sl].reshape(BC * A, D),
                "gidx": _make_gidx(indices[sl, 0]),
            }
        )

    res = run_bass_kernel_spmd(nc, in_maps, core_ids=list(range(N_CORES)))
    LAST_RESULTS = res

    out = np.empty((B, D), dtype=np.float32)
    for c in range(N_CORES):
        out[c * BC : (c + 1) * BC] = res.results[c]["out"].reshape(BC, D)
    return out
